# revision 36
# baseline (speedup 1.0000x reference)
"""Multihead attention (B=2, S=2048, E=1024, H=16) on 8 TRN2 cores.

Sharding (hybrid data/tensor parallel): core c handles batch c//4 and heads
4g..4g+3 where g = c%4 — each core projects a 256-column slice of Q/K/V for
its batch, runs attention for its 4 heads, and produces its partial
contribution to the output projection.  The host sums 4 partials per batch
and adds the output bias.  Inputs per core are 12 MB (x^T of one batch,
bf16) instead of 24 MB for pure head-parallel — DMA is halved.

Per-core program:
  x^T [E, S] bf16 is DMA'd up-front into persistent SBUF (96 seq-major
  chunk DMAs issued at kernel start) so projections are pure PE work and
  never wait on just-in-time transfers.  QKV projections contract E on the
  partition dim producing Q^T/K^T/V^T [128, 2(pair), S] (partition =
  within-head-pair dim).  Projection bias is added on the scalar engine
  (Identity activation, per-partition bias AP) since ACT is idle during
  projections.  V^T is re-transposed to [kpos, d] chunks with a trailing
  ones column ([V | 1]) so the softmax denominator falls out of the PV
  matmul (row 64 of the ctx PSUM tile).

  Attention per (head-pair, q-block 512): for each of 16 kpos tiles the two
  heads' score matmuls (K=64 contraction, base partitions 0/64 → PE row
  tiles (0,0)/(64,0), concurrent in the array) write the two halves of one
  [128, 1024] PSUM tile spanning 2 banks; ONE scalar-engine Exp covers both
  heads, halving ACT instruction count — ACT is the critical engine.  The
  emission runs scores one kpos tile ahead of the PV matmuls so the PE
  FIFO never head-of-line blocks on ACT.  Each unit's softmax
  normalization (reciprocal + PE-replicated row + DVE multiply into ctxT)
  is DEFERRED into the next unit, emitted right after its first score
  matmul, so the serial DVE→PE→DVE chain never stalls the exp stream.

  The output projection (f32r, full-rate at N=512) is cut into 16 s-tile
  pieces interleaved into later attention units as PE fillers; projection
  of q-block qb+1 is likewise spread through attention of qb.  PSUM
  budget: sc 2x2 banks + cx 2 (ctx tiles and deferred-norm rrep share the
  pool) + pp 2 = 8 exactly.
"""

import numpy as np
import ml_dtypes

# Problem constants (hardcoded per the task contract).
B, S, E, H = 2, 2048, 1024, 16
D = E // H          # 64
NCORES = 8
GPB = 4             # head-groups (cores) per batch
DOUT = E // GPB     # 256 = 4 heads x 64 per core
KE = E // 128       # 8 contraction tiles over E
SEQT = 512          # seq tile for projections / q-block for attention
QB = S // SEQT      # 4 q-blocks
KT = S // 128       # 16 kpos tiles
ISD = float(D) ** -0.5

_PROGRAM = None


# ---------------------------------------------------------------------------
# Workarounds for this walrus build: at most ONE sync wait per instruction is
# reliably accepted ("Too many sync wait commands").  (1) tile's final drain
# gets one wait per logical proc — split them over single-wait SP NOPs;
# (2) a general post-pass moves any instruction's excess waits onto
# preceding same-engine NOPs (engine program order preserves semantics).
# ---------------------------------------------------------------------------


def _install_tile_drain_patch():
    import concourse.mybir as mybir
    import concourse.tile as tile
    from concourse.tile import ScopedClock

    if getattr(tile.TileContext, "_drain_patch_installed", False):
        return

    def _patched_drain_and_barrier(self, tick_clock, wait_clock):
        nc = self.nc
        carrier = nc.sync.nop(nofuse=True)
        wait_clock.add_sem_waits(
            carrier.ins, ScopedClock({None: tick_clock.global_clock})
        )
        si = carrier.ins.sync_info
        waits = list(si.on_wait) if si and si.on_wait else []
        ups = list(si.on_update) if si and si.on_update else []
        if len(waits) > 1:
            carrier.ins.sync_info = mybir.SyncInfo(on_wait=[waits[0]], on_update=ups)
            for w in waits[1:]:
                n2 = nc.sync.nop(nofuse=True)
                n2.ins.sync_info = mybir.SyncInfo(on_wait=[w], on_update=[])
        nc.sync.drain()
        nc.all_engine_barrier()
        popped = nc._tile_sem_poison_stack.pop()
        assert popped is self._sem_poison
        nc.clear_and_free_semaphores(list(self.sems.allocated().values()))
        nc.all_engine_barrier()

    tile.TileContext._drain_and_barrier = _patched_drain_and_barrier
    tile.TileContext._drain_patch_installed = True


MAX_WAITS = 1


def _split_excess_waits(nc):
    import concourse.mybir as mybir

    for bb in nc.main_func.blocks:
        il = list(bb.instructions)
        out = []
        changed = False
        for ins in il:
            si = ins.sync_info
            waits = list(si.on_wait) if si and si.on_wait else []
            if len(waits) > MAX_WAITS:
                changed = True
                extras = waits[: len(waits) - MAX_WAITS]
                keep = waits[len(extras):]
                for i in range(0, len(extras), MAX_WAITS):
                    chunk = extras[i : i + MAX_WAITS]
                    nop = mybir.InstNoOp(
                        name=nc.get_next_instruction_name(), ins=[], outs=[]
                    )
                    nop.engine = ins.engine
                    nop.sync_info = mybir.SyncInfo(on_wait=chunk, on_update=[])
                    out.append(nop)
                ins.sync_info = mybir.SyncInfo(
                    on_wait=keep, on_update=list(si.on_update) if si.on_update else []
                )
            out.append(ins)
        if changed:
            bb.instructions = out
    nc._waits_split = True


def _build_program():
    import concourse.bass as bass
    import concourse.mybir as mybir
    import concourse.tile as tile
    from concourse.masks import make_identity

    _install_tile_drain_patch()

    f32 = mybir.dt.float32
    f32r = mybir.dt.float32r
    bf16 = mybir.dt.bfloat16
    AF = mybir.ActivationFunctionType

    nc = bass.Bass("TRN2", target_bir_lowering=False, debug=False)

    # DRAM I/O (per core).
    xq = nc.dram_tensor("xq", [KE, 128, S], bf16, kind="ExternalInput").ap()
    xk = nc.dram_tensor("xk", [KE, 128, S], bf16, kind="ExternalInput").ap()
    xv = nc.dram_tensor("xv", [KE, 128, S], bf16, kind="ExternalInput").ap()
    wq = nc.dram_tensor("wq", [KE, 128, DOUT], bf16, kind="ExternalInput").ap()
    wk = nc.dram_tensor("wk", [KE, 128, DOUT], bf16, kind="ExternalInput").ap()
    wv = nc.dram_tensor("wv", [KE, 128, DOUT], bf16, kind="ExternalInput").ap()
    wo = nc.dram_tensor("wo", [128, 2, E], bf16, kind="ExternalInput").ap()
    bq = nc.dram_tensor("bq", [128, 2], f32, kind="ExternalInput").ap()
    bk = nc.dram_tensor("bk", [128, 2], f32, kind="ExternalInput").ap()
    bv = nc.dram_tensor("bv", [128, 2], f32, kind="ExternalInput").ap()
    out = nc.dram_tensor("out", [S, E], bf16, kind="ExternalOutput").ap()

    with tile.TileContext(nc) as tc:
        with (
            nc.allow_low_precision(reason="bf16/f32r attention pipeline"),
            # PSUM pools in declaration order → bank-aligned offsets:
            # sc 2x2 banks @0, cx 2x1 @8KB, pp 2x1 @12KB = 16KB exactly.
            tc.tile_pool(name="sc_ps", bufs=2, space="PSUM") as sc_ps,
            tc.tile_pool(name="cx_ps", bufs=2, space="PSUM") as cx_ps,
            tc.tile_pool(name="pp_ps", bufs=2, space="PSUM") as pp_ps,
            tc.tile_pool(name="consts", bufs=1) as consts,
            tc.tile_pool(name="persist", bufs=1) as persist,
            tc.tile_pool(name="ptp", bufs=9) as ptp,
            tc.tile_pool(name="outp", bufs=2) as outp,
            tc.tile_pool(name="small", bufs=2) as small,
        ):
            # ---- constants / persistent SBUF state ----
            ident_f32 = consts.tile([128, 128], f32)
            make_identity(nc, ident_f32[:])
            ident = consts.tile([128, 128], bf16)
            nc.vector.tensor_copy(ident[:], ident_f32[:])
            onesf = consts.tile([128, 1], f32)
            nc.vector.memset(onesf[:], 1.0)


            w_sb = {}
            b_sb = {}
            for name in ("q", "k", "v"):
                w_sb[name] = persist.tile(
                    [128, KE, DOUT], bf16, tag=f"w{name}", name=f"w{name}_sb"
                )
                b_sb[name] = persist.tile(
                    [128, 2], f32, tag=f"b{name}", name=f"b{name}_sb"
                )
            wo_sb = persist.tile([128, 2, E], bf16, tag="wo")

            # x^T preloaded into SBUF.  DMA bandwidth here is ~110 GB/s per
            # core, so the 12 MB input stream IS the critical path of the
            # first half of the kernel: transfer K and V seq-tile-major
            # (so attention ladders behind the stream), q-block 0 of Q
            # early, the rest of Q last.  Issues are split across the sync
            # and scalar engines (~0.6us per dma_start issue cost).
            xdram = {"q": xq, "k": xk, "v": xv}
            x_sb = {}
            for name in ("k", "v", "q"):
                x_sb[name] = persist.tile(
                    [128, KE, S], bf16, tag=f"x{name}", name=f"x{name}_sb"
                )
            # scalar queue: q st0, then q weights, then the rest of q
            for k in range(KE):
                nc.scalar.dma_start(x_sb["q"][:, k, 0:SEQT], xq[k, :, 0:SEQT])
            for k in range(KE):
                nc.scalar.dma_start(w_sb["q"][:, k, :], wq[k])
            nc.scalar.dma_start(b_sb["q"][:], bq[:])
            # gpsimd queue: the whole V stream + its weights
            for k in range(KE):
                nc.gpsimd.dma_start(x_sb["v"][:, k, 0:SEQT], xv[k, :, 0:SEQT])
            for k in range(KE):
                nc.gpsimd.dma_start(w_sb["v"][:, k, :], wv[k])
            nc.gpsimd.dma_start(b_sb["v"][:], bv[:])
            for st in range(1, QB):
                sl = bass.ts(st, SEQT)
                for k in range(KE):
                    nc.gpsimd.dma_start(x_sb["v"][:, k, sl], xv[k, :, sl])
            # sync queue: K stream + its weights + wo
            for k in range(KE):
                nc.sync.dma_start(x_sb["k"][:, k, 0:SEQT], xk[k, :, 0:SEQT])
            for k in range(KE):
                nc.sync.dma_start(w_sb["k"][:, k, :], wk[k])
            nc.sync.dma_start(b_sb["k"][:], bk[:])
            for st in range(1, QB):
                sl = bass.ts(st, SEQT)
                for k in range(KE):
                    nc.sync.dma_start(x_sb["k"][:, k, sl], xk[k, :, sl])
            nc.sync.dma_start(wo_sb[:], wo[:])
            for st in range(1, QB):
                sl = bass.ts(st, SEQT)
                for k in range(KE):
                    nc.scalar.dma_start(x_sb["q"][:, k, sl], xq[k, :, sl])

            qt_sb = persist.tile([128, 2, S], bf16, tag="qt")
            kt_sb = persist.tile([128, 2, S], bf16, tag="kt")
            vt_sb = persist.tile([128, 2, S], bf16, tag="vt")
            xT_sb = {"q": qt_sb, "k": kt_sb, "v": vt_sb}
            # [V | ones] per (kpos chunk, head): [128, 16, 4, 65] bf16
            v_sb = persist.tile([128, KT, 4, D + 1], bf16, tag="vn")
            nc.vector.tensor_copy(
                v_sb[:, :, :, D], onesf[:, 0:1].broadcast_to([128, KT, 4])
            )
            ctxT_sb = persist.tile([128, 2, S], bf16, tag="ctxT")

            def proj_mms(name, st):
                """One seq-512 projection step: 8 x chunks x 2 dout halves
                accumulating into 2 pp PSUM tiles, then ACT bias-adds."""
                sl = bass.ts(st, SEQT)
                ps = [
                    pp_ps.tile([128, SEQT], f32, tag="pp", name=f"ps{name}{st}{j}")
                    for j in range(2)
                ]
                for k in range(KE):
                    for j in range(2):
                        nc.tensor.matmul(
                            ps[j][:],
                            lhsT=w_sb[name][:, k, bass.ts(j, 128)],
                            rhs=x_sb[name][:, k, sl],
                            start=(k == 0),
                            stop=(k == KE - 1),
                        )
                for j in range(2):
                    nc.vector.tensor_scalar_add(
                        xT_sb[name][:, j, sl], ps[j][:], b_sb[name][:, j : j + 1]
                    )

            def v_transposes(st):
                """Transpose this seq slice of V^T into [V | ones] chunks
                (PE transpose + ACT copies)."""
                for hp in range(2):
                    for ci in range(4 * st, 4 * st + 4):
                        tp = pp_ps.tile([128, 128], bf16, tag="pp", name="tp")
                        nc.tensor.transpose(
                            tp[:], vt_sb[:, hp, bass.ts(ci, 128)], ident[:]
                        )
                        for e in range(2):
                            nc.vector.tensor_copy(
                                v_sb[:, ci, 2 * hp + e, 0:D], tp[:, bass.ts(e, D)]
                            )

            def outproj_piece(m):
                """Partial out-proj for s-tile m: 2 e-halves x 2 ctx chunks,
                DVE PSUM→SBUF copies, DMA to DRAM."""
                ob = outp.tile([128, E], bf16, tag="ob", name="ob")
                msl = bass.ts(m, 128)
                for et in range(2):
                    ops = pp_ps.tile([128, SEQT], f32, tag="pp", name="ops")
                    for j in range(2):
                        nc.tensor.matmul(
                            ops[:],
                            lhsT=ctxT_sb[:, j, msl],
                            rhs=wo_sb[:, j, bass.ts(et, SEQT)],
                            start=(j == 0),
                            stop=(j == 1),
                        )
                    nc.vector.tensor_copy(ob[:, bass.ts(et, SEQT)], ops[:])
                nc.sync.dma_start(out[msl, :], ob[:])

            def emit_norm(hp, qb, ctx):
                """Softmax normalization of a finished unit's ctx tiles into
                ctxT.  The reciprocal runs on ACT as exp(-ln(den)) — ~0.6us
                per head vs 3.3us for the DVE reciprocal, and ln/exp share
                one activation table.  DVE then broadcasts the reciprocal
                row across partitions with stream_shuffle and multiplies
                straight out of ctx PSUM into ctxT bf16."""
                qsl = bass.ts(qb, SEQT)
                rec = small.tile([64, SEQT], f32, tag="reco", name="reco")
                for e in range(2):
                    lnt = small.tile([1, SEQT], f32, tag=f"ln{e}", name=f"ln{e}")
                    nc.scalar.activation(lnt[:], ctx[e][D : D + 1, :], AF.Ln)
                    nc.scalar.activation(
                        rec[32 * e : 32 * e + 1, :], lnt[:], AF.Exp, scale=-1.0
                    )
                rr = [
                    small.tile([D, SEQT], f32, tag=f"rr{e}", name=f"rr{e}")
                    for e in range(2)
                ]
                for e in range(2):
                    for half in range(2):
                        nc.vector.stream_shuffle(
                            rr[e][32 * half : 32 * half + 32, :],
                            rec[32 * e : 32 * e + 32, :],
                            mask=[0] * 32,
                        )
                for e in range(2):
                    nc.vector.tensor_tensor(
                        out=ctxT_sb[slice(64 * e, 64 * e + 64), hp, qsl],
                        in0=ctx[e][0:D, :],
                        in1=rr[e][:],
                        op=mybir.AluOpType.mult,
                    )

            def attn_unit(hp, qb, prev=None, fillers=None):
                """One (head-pair, q-block) attention unit.  Scores run one
                kpos tile ahead of PV so PE never blocks on ACT.  prev =
                (hp', qb', ctx') of the previous unit — its normalization is
                emitted after this unit's first exp.  fillers: {t: callable}
                emitted at the START of iteration t (before sc_{t+1}, so a
                filler's PE work can never sit behind a score matmul that
                depends on the filler's own output)."""
                qsl = bass.ts(qb, SEQT)
                fillers = fillers or {}

                def sc_mms(t):
                    sc = sc_ps.tile([128, 2 * SEQT], f32, tag="sc", name=f"sct{t}")
                    ksl = bass.ts(t, 128)
                    for e in range(2):
                        esl = slice(64 * e, 64 * e + 64)
                        nc.tensor.matmul(
                            sc[:, bass.ts(e, SEQT)],
                            lhsT=kt_sb[esl, hp, ksl],
                            rhs=qt_sb[esl, hp, qsl],
                            start=True,
                            stop=True,
                        )
                    return sc

                ctx = [
                    cx_ps.tile([D + 1, SEQT], f32, tag="cx", name=f"ctx{e}")
                    for e in range(2)
                ]

                sc_next = sc_mms(0)
                for t in range(KT):
                    if t in fillers:
                        fillers[t]()
                    sc_cur = sc_next
                    if t + 1 < KT:
                        sc_next = sc_mms(t + 1)
                    pt = ptp.tile([128, 2 * SEQT], bf16, tag="pt", name="pt")
                    nc.scalar.activation(pt[:], sc_cur[:], AF.Exp, scale=ISD)
                    if t == 1 and prev is not None:
                        emit_norm(*prev)
                    for e in range(2):
                        nc.tensor.matmul(
                            ctx[e][:],
                            lhsT=v_sb[:, t, 2 * hp + e, :],
                            rhs=pt[:, bass.ts(e, SEQT)],
                            start=(t == 0),
                            stop=(t == KT - 1),
                        )
                return (hp, qb, ctx)

            def proj_kv(st):
                def cb():
                    proj_mms("k", st)
                    proj_mms("v", st)
                    v_transposes(st)
                return cb

            def proj_q(st):
                return lambda: proj_mms("q", st)

            def piece(m):
                return lambda: outproj_piece(m)

            # ---- emission ----
            # Head: K/V/Q projections for seq-tile 0 right behind the input
            # stream; later seq-tiles ladder INSIDE attention unit (0,0) as
            # fillers timed to the stream (sc_t for t>=4s needs kv st s).
            proj_mms("k", 0)
            proj_mms("v", 0)
            v_transposes(0)
            proj_mms("q", 0)

            u = attn_unit(0, 0, fillers={3: proj_kv(1), 7: proj_kv(2),
                                         11: proj_kv(3)})
            u = attn_unit(1, 0, prev=u, fillers={3: proj_q(1)})
            u = attn_unit(0, 1, prev=u, fillers={3: proj_q(2)})
            u = attn_unit(1, 1, prev=u, fillers={2: piece(0), 6: piece(1),
                                                 10: piece(2)})
            u = attn_unit(0, 2, prev=u, fillers={2: piece(3), 6: proj_q(3)})
            u = attn_unit(1, 2, prev=u, fillers={2: piece(4), 6: piece(5),
                                                 10: piece(6), 13: piece(7)})
            u = attn_unit(0, 3, prev=u, fillers={2: piece(8), 6: piece(9)})
            u = attn_unit(1, 3, prev=u, fillers={2: piece(10), 6: piece(11)})
            emit_norm(*u)
            for m in range(12, 16):
                outproj_piece(m)

    return nc


def _get_program():
    global _PROGRAM
    if _PROGRAM is None:
        _PROGRAM = _build_program()
        if not getattr(_PROGRAM, "_waits_split", False):
            _split_excess_waits(_PROGRAM)
    return _PROGRAM


def kernel(query, key, value, Wq, bq, Wk, bk, Wv, bv, Wo, bo):
    from concourse.bass_utils import run_bass_kernel_spmd

    nc = _get_program()

    bf = ml_dtypes.bfloat16
    q3 = np.asarray(query, np.float32)
    k3 = np.asarray(key, np.float32)
    v3 = np.asarray(value, np.float32)
    # per-batch x^T [E, S] -> [KE, 128, S], rounded to bf16 on host (the
    # bf16 matmul rounds its inputs anyway)
    xs = {}
    for b in range(B):
        xs[b] = {
            "xq": np.ascontiguousarray(q3[b].T).astype(bf).reshape(KE, 128, S),
            "xk": np.ascontiguousarray(k3[b].T).astype(bf).reshape(KE, 128, S),
            "xv": np.ascontiguousarray(v3[b].T).astype(bf).reshape(KE, 128, S),
        }

    Wq = np.asarray(Wq, np.float32)
    Wk = np.asarray(Wk, np.float32)
    Wv = np.asarray(Wv, np.float32)
    Wo = np.asarray(Wo, np.float32)
    bqf = np.asarray(bq, np.float32)
    bkf = np.asarray(bk, np.float32)
    bvf = np.asarray(bv, np.float32)

    wmaps = []
    for g in range(GPB):
        rsl = slice(DOUT * g, DOUT * (g + 1))
        wmaps.append(
            {
                # lhsT for the projections: (W_g)^T [E, DOUT] -> [KE,128,DOUT]
                "wq": np.ascontiguousarray(Wq[rsl, :].T).astype(bf).reshape(KE, 128, DOUT),
                "wk": np.ascontiguousarray(Wk[rsl, :].T).astype(bf).reshape(KE, 128, DOUT),
                "wv": np.ascontiguousarray(Wv[rsl, :].T).astype(bf).reshape(KE, 128, DOUT),
                # out-proj rhs: Wo^T rows rsl as [128, 2, E], bf16
                "wo": np.ascontiguousarray(
                    Wo[:, rsl].T.reshape(2, 128, E).transpose(1, 0, 2)
                ).astype(bf),
                "bq": np.ascontiguousarray(bqf[rsl].reshape(2, 128).T),
                "bk": np.ascontiguousarray(bkf[rsl].reshape(2, 128).T),
                "bv": np.ascontiguousarray(bvf[rsl].reshape(2, 128).T),
            }
        )

    in_maps = []
    for c in range(NCORES):
        b, g = c // GPB, c % GPB
        m = dict(xs[b])
        m.update(wmaps[g])
        in_maps.append(m)

    res = run_bass_kernel_spmd(nc, in_maps, list(range(NCORES)), trace=False)
    bof = np.asarray(bo, np.float32)
    full = np.empty((B, S, E), np.float32)
    for b in range(B):
        acc = res.results[b * GPB]["out"].astype(np.float32)
        for g in range(1, GPB):
            acc += res.results[b * GPB + g]["out"].astype(np.float32)
        full[b] = acc + bof[None, :]
    return full


# revision 38
# speedup vs baseline: 1.1818x; 1.1818x over previous
"""Multihead attention (B=2, S=2048, E=1024, H=16) on 8 TRN2 cores.

Sharding (hybrid data/tensor parallel): core c handles batch c//4 and heads
4g..4g+3 where g = c%4 — each core projects a 256-column slice of Q/K/V for
its batch, runs attention for its 4 heads, and produces its partial
contribution to the output projection.  The host sums 4 partials per batch
and adds the output bias.  Inputs per core are 12 MB (x^T of one batch,
bf16) instead of 24 MB for pure head-parallel — DMA is halved.

Per-core program:
  x^T [E, S] bf16 is DMA'd up-front into persistent SBUF (96 seq-major
  chunk DMAs issued at kernel start) so projections are pure PE work and
  never wait on just-in-time transfers.  QKV projections contract E on the
  partition dim producing Q^T/K^T/V^T [128, 2(pair), S] (partition =
  within-head-pair dim).  Projection bias is added on the scalar engine
  (Identity activation, per-partition bias AP) since ACT is idle during
  projections.  V^T is re-transposed to [kpos, d] chunks with a trailing
  ones column ([V | 1]) so the softmax denominator falls out of the PV
  matmul (row 64 of the ctx PSUM tile).

  Attention per (head-pair, q-block 512): for each of 16 kpos tiles the two
  heads' score matmuls (K=64 contraction, base partitions 0/64 → PE row
  tiles (0,0)/(64,0), concurrent in the array) write the two halves of one
  [128, 1024] PSUM tile spanning 2 banks; ONE scalar-engine Exp covers both
  heads, halving ACT instruction count — ACT is the critical engine.  The
  emission runs scores one kpos tile ahead of the PV matmuls so the PE
  FIFO never head-of-line blocks on ACT.  Each unit's softmax
  normalization (reciprocal + PE-replicated row + DVE multiply into ctxT)
  is DEFERRED into the next unit, emitted right after its first score
  matmul, so the serial DVE→PE→DVE chain never stalls the exp stream.

  The output projection (f32r, full-rate at N=512) is cut into 16 s-tile
  pieces interleaved into later attention units as PE fillers; projection
  of q-block qb+1 is likewise spread through attention of qb.  PSUM
  budget: sc 2x2 banks + cx 2 (ctx tiles and deferred-norm rrep share the
  pool) + pp 2 = 8 exactly.
"""

import numpy as np
import ml_dtypes

# Problem constants (hardcoded per the task contract).
B, S, E, H = 2, 2048, 1024, 16
D = E // H          # 64
NCORES = 8
GPB = 4             # head-groups (cores) per batch
DOUT = E // GPB     # 256 = 4 heads x 64 per core
KE = E // 128       # 8 contraction tiles over E
SEQT = 512          # seq tile for projections / q-block for attention
QB = S // SEQT      # 4 q-blocks
KT = S // 128       # 16 kpos tiles
ISD = float(D) ** -0.5

_PROGRAM = None


# ---------------------------------------------------------------------------
# Workarounds for this walrus build: at most ONE sync wait per instruction is
# reliably accepted ("Too many sync wait commands").  (1) tile's final drain
# gets one wait per logical proc — split them over single-wait SP NOPs;
# (2) a general post-pass moves any instruction's excess waits onto
# preceding same-engine NOPs (engine program order preserves semantics).
# ---------------------------------------------------------------------------


def _install_tile_drain_patch():
    import concourse.mybir as mybir
    import concourse.tile as tile
    from concourse.tile import ScopedClock

    if getattr(tile.TileContext, "_drain_patch_installed", False):
        return

    def _patched_drain_and_barrier(self, tick_clock, wait_clock):
        nc = self.nc
        carrier = nc.sync.nop(nofuse=True)
        wait_clock.add_sem_waits(
            carrier.ins, ScopedClock({None: tick_clock.global_clock})
        )
        si = carrier.ins.sync_info
        waits = list(si.on_wait) if si and si.on_wait else []
        ups = list(si.on_update) if si and si.on_update else []
        if len(waits) > 1:
            carrier.ins.sync_info = mybir.SyncInfo(on_wait=[waits[0]], on_update=ups)
            for w in waits[1:]:
                n2 = nc.sync.nop(nofuse=True)
                n2.ins.sync_info = mybir.SyncInfo(on_wait=[w], on_update=[])
        nc.sync.drain()
        nc.all_engine_barrier()
        popped = nc._tile_sem_poison_stack.pop()
        assert popped is self._sem_poison
        nc.clear_and_free_semaphores(list(self.sems.allocated().values()))
        nc.all_engine_barrier()

    tile.TileContext._drain_and_barrier = _patched_drain_and_barrier
    tile.TileContext._drain_patch_installed = True


MAX_WAITS = 1


def _split_excess_waits(nc):
    import concourse.mybir as mybir

    for bb in nc.main_func.blocks:
        il = list(bb.instructions)
        out = []
        changed = False
        for ins in il:
            si = ins.sync_info
            waits = list(si.on_wait) if si and si.on_wait else []
            if len(waits) > MAX_WAITS:
                changed = True
                extras = waits[: len(waits) - MAX_WAITS]
                keep = waits[len(extras):]
                for i in range(0, len(extras), MAX_WAITS):
                    chunk = extras[i : i + MAX_WAITS]
                    nop = mybir.InstNoOp(
                        name=nc.get_next_instruction_name(), ins=[], outs=[]
                    )
                    nop.engine = ins.engine
                    nop.sync_info = mybir.SyncInfo(on_wait=chunk, on_update=[])
                    out.append(nop)
                ins.sync_info = mybir.SyncInfo(
                    on_wait=keep, on_update=list(si.on_update) if si.on_update else []
                )
            out.append(ins)
        if changed:
            bb.instructions = out
    nc._waits_split = True


def _build_program():
    import concourse.bass as bass
    import concourse.mybir as mybir
    import concourse.tile as tile
    from concourse.masks import make_identity

    _install_tile_drain_patch()

    f32 = mybir.dt.float32
    f32r = mybir.dt.float32r
    bf16 = mybir.dt.bfloat16
    AF = mybir.ActivationFunctionType

    nc = bass.Bass("TRN2", target_bir_lowering=False, debug=False)

    # DRAM I/O (per core).
    xq = nc.dram_tensor("xq", [KE, 128, S], bf16, kind="ExternalInput").ap()
    xk = nc.dram_tensor("xk", [KE, 128, S], bf16, kind="ExternalInput").ap()
    xv = nc.dram_tensor("xv", [KE, 128, S], bf16, kind="ExternalInput").ap()
    wq = nc.dram_tensor("wq", [KE, 128, DOUT], bf16, kind="ExternalInput").ap()
    wk = nc.dram_tensor("wk", [KE, 128, DOUT], bf16, kind="ExternalInput").ap()
    wv = nc.dram_tensor("wv", [KE, 128, DOUT], bf16, kind="ExternalInput").ap()
    wo = nc.dram_tensor("wo", [128, 2, E], bf16, kind="ExternalInput").ap()
    bq = nc.dram_tensor("bq", [128, 2], f32, kind="ExternalInput").ap()
    bk = nc.dram_tensor("bk", [128, 2], f32, kind="ExternalInput").ap()
    bv = nc.dram_tensor("bv", [128, 2], f32, kind="ExternalInput").ap()
    out = nc.dram_tensor("out", [S, E], bf16, kind="ExternalOutput").ap()

    with tile.TileContext(nc) as tc:
        with (
            nc.allow_low_precision(reason="bf16/f32r attention pipeline"),
            # PSUM pools in declaration order → bank-aligned offsets:
            # sc 2x2 banks @0, cx 2x1 @8KB, pp 2x1 @12KB = 16KB exactly.
            tc.tile_pool(name="sc_ps", bufs=2, space="PSUM") as sc_ps,
            tc.tile_pool(name="cx_ps", bufs=2, space="PSUM") as cx_ps,
            tc.tile_pool(name="pp_ps", bufs=2, space="PSUM") as pp_ps,
            tc.tile_pool(name="consts", bufs=1) as consts,
            tc.tile_pool(name="persist", bufs=1) as persist,
            tc.tile_pool(name="ptp", bufs=9) as ptp,
            tc.tile_pool(name="outp", bufs=2) as outp,
            tc.tile_pool(name="small", bufs=2) as small,
        ):
            # ---- constants / persistent SBUF state ----
            ident_f32 = consts.tile([128, 128], f32)
            make_identity(nc, ident_f32[:])
            ident = consts.tile([128, 128], bf16)
            nc.vector.tensor_copy(ident[:], ident_f32[:])
            onesf = consts.tile([128, 1], f32)
            nc.vector.memset(onesf[:], 1.0)


            w_sb = {}
            b_sb = {}
            for name in ("q", "k", "v"):
                w_sb[name] = persist.tile(
                    [128, KE, DOUT], bf16, tag=f"w{name}", name=f"w{name}_sb"
                )
                b_sb[name] = persist.tile(
                    [128, 2], f32, tag=f"b{name}", name=f"b{name}_sb"
                )
            wo_sb = persist.tile([128, 2, E], bf16, tag="wo")

            # x^T preloaded into SBUF.  DMA bandwidth here is ~110 GB/s per
            # core, so the 12 MB input stream IS the critical path of the
            # first half of the kernel: transfer K and V seq-tile-major
            # (so attention ladders behind the stream), q-block 0 of Q
            # early, the rest of Q last.  Issues are split across the sync
            # and scalar engines (~0.6us per dma_start issue cost).
            xdram = {"q": xq, "k": xk, "v": xv}
            x_sb = {}
            for name in ("k", "v", "q"):
                x_sb[name] = persist.tile(
                    [128, KE, S], bf16, tag=f"x{name}", name=f"x{name}_sb"
                )
            # scalar queue: q st0, then q weights, then the rest of q
            for k in range(KE):
                nc.scalar.dma_start(x_sb["q"][:, k, 0:SEQT], xq[k, :, 0:SEQT])
            for k in range(KE):
                nc.scalar.dma_start(w_sb["q"][:, k, :], wq[k])
            nc.scalar.dma_start(b_sb["q"][:], bq[:])
            # sync queue: K st0, K weights, V st0, V weights, later K/V waves
            for k in range(KE):
                nc.sync.dma_start(x_sb["k"][:, k, 0:SEQT], xk[k, :, 0:SEQT])
            for k in range(KE):
                nc.sync.dma_start(w_sb["k"][:, k, :], wk[k])
            nc.sync.dma_start(b_sb["k"][:], bk[:])
            for k in range(KE):
                nc.sync.dma_start(x_sb["v"][:, k, 0:SEQT], xv[k, :, 0:SEQT])
            for k in range(KE):
                nc.sync.dma_start(w_sb["v"][:, k, :], wv[k])
            nc.sync.dma_start(b_sb["v"][:], bv[:])
            for st in range(1, QB):
                sl = bass.ts(st, SEQT)
                for k in range(KE):
                    nc.sync.dma_start(x_sb["k"][:, k, sl], xk[k, :, sl])
                for k in range(KE):
                    nc.sync.dma_start(x_sb["v"][:, k, sl], xv[k, :, sl])
            nc.sync.dma_start(wo_sb[:], wo[:])
            for st in range(1, QB):
                sl = bass.ts(st, SEQT)
                for k in range(KE):
                    nc.scalar.dma_start(x_sb["q"][:, k, sl], xq[k, :, sl])

            qt_sb = persist.tile([128, 2, S], bf16, tag="qt")
            kt_sb = persist.tile([128, 2, S], bf16, tag="kt")
            vt_sb = persist.tile([128, 2, S], bf16, tag="vt")
            xT_sb = {"q": qt_sb, "k": kt_sb, "v": vt_sb}
            # [V | ones] per (kpos chunk, head): [128, 16, 4, 65] bf16
            v_sb = persist.tile([128, KT, 4, D + 1], bf16, tag="vn")
            nc.vector.tensor_copy(
                v_sb[:, :, :, D], onesf[:, 0:1].broadcast_to([128, KT, 4])
            )
            ctxT_sb = persist.tile([128, 2, S], bf16, tag="ctxT")

            def proj_mms(name, st):
                """One seq-512 projection step: 8 x chunks x 2 dout halves
                accumulating into 2 pp PSUM tiles, then ACT bias-adds."""
                sl = bass.ts(st, SEQT)
                ps = [
                    pp_ps.tile([128, SEQT], f32, tag="pp", name=f"ps{name}{st}{j}")
                    for j in range(2)
                ]
                for k in range(KE):
                    for j in range(2):
                        nc.tensor.matmul(
                            ps[j][:],
                            lhsT=w_sb[name][:, k, bass.ts(j, 128)],
                            rhs=x_sb[name][:, k, sl],
                            start=(k == 0),
                            stop=(k == KE - 1),
                        )
                for j in range(2):
                    nc.vector.tensor_scalar_add(
                        xT_sb[name][:, j, sl], ps[j][:], b_sb[name][:, j : j + 1]
                    )

            def v_transposes(st):
                """Transpose this seq slice of V^T into [V | ones] chunks
                (PE transpose + ACT copies)."""
                for hp in range(2):
                    for ci in range(4 * st, 4 * st + 4):
                        tp = pp_ps.tile([128, 128], bf16, tag="pp", name="tp")
                        nc.tensor.transpose(
                            tp[:], vt_sb[:, hp, bass.ts(ci, 128)], ident[:]
                        )
                        for e in range(2):
                            nc.vector.tensor_copy(
                                v_sb[:, ci, 2 * hp + e, 0:D], tp[:, bass.ts(e, D)]
                            )

            def outproj_piece(m):
                """Partial out-proj for s-tile m: 2 e-halves x 2 ctx chunks,
                DVE PSUM→SBUF copies, DMA to DRAM."""
                ob = outp.tile([128, E], bf16, tag="ob", name="ob")
                msl = bass.ts(m, 128)
                for et in range(2):
                    ops = pp_ps.tile([128, SEQT], f32, tag="pp", name="ops")
                    for j in range(2):
                        nc.tensor.matmul(
                            ops[:],
                            lhsT=ctxT_sb[:, j, msl],
                            rhs=wo_sb[:, j, bass.ts(et, SEQT)],
                            start=(j == 0),
                            stop=(j == 1),
                        )
                    nc.vector.tensor_copy(ob[:, bass.ts(et, SEQT)], ops[:])
                nc.sync.dma_start(out[msl, :], ob[:])

            def emit_norm(hp, qb, ctx):
                """Softmax normalization of a finished unit's ctx tiles into
                ctxT.  The reciprocal runs on ACT as exp(-ln(den)) — ~0.6us
                per head vs 3.3us for the DVE reciprocal, and ln/exp share
                one activation table.  DVE then broadcasts the reciprocal
                row across partitions with stream_shuffle and multiplies
                straight out of ctx PSUM into ctxT bf16."""
                qsl = bass.ts(qb, SEQT)
                rec = small.tile([64, SEQT], f32, tag="reco", name="reco")
                for e in range(2):
                    lnt = small.tile([1, SEQT], f32, tag=f"ln{e}", name=f"ln{e}")
                    nc.scalar.activation(lnt[:], ctx[e][D : D + 1, :], AF.Ln)
                    nc.scalar.activation(
                        rec[32 * e : 32 * e + 1, :], lnt[:], AF.Exp, scale=-1.0
                    )
                rr = [
                    small.tile([D, SEQT], f32, tag=f"rr{e}", name=f"rr{e}")
                    for e in range(2)
                ]
                for e in range(2):
                    for half in range(2):
                        nc.vector.stream_shuffle(
                            rr[e][32 * half : 32 * half + 32, :],
                            rec[32 * e : 32 * e + 32, :],
                            mask=[0] * 32,
                        )
                for e in range(2):
                    nc.vector.tensor_tensor(
                        out=ctxT_sb[slice(64 * e, 64 * e + 64), hp, qsl],
                        in0=ctx[e][0:D, :],
                        in1=rr[e][:],
                        op=mybir.AluOpType.mult,
                    )

            def attn_unit(hp, qb, prev=None, fillers=None):
                """One (head-pair, q-block) attention unit.  Scores run one
                kpos tile ahead of PV so PE never blocks on ACT.  prev =
                (hp', qb', ctx') of the previous unit — its normalization is
                emitted after this unit's first exp.  fillers: {t: callable}
                emitted at the START of iteration t (before sc_{t+1}, so a
                filler's PE work can never sit behind a score matmul that
                depends on the filler's own output)."""
                qsl = bass.ts(qb, SEQT)
                fillers = fillers or {}

                def sc_mms(t):
                    sc = sc_ps.tile([128, 2 * SEQT], f32, tag="sc", name=f"sct{t}")
                    ksl = bass.ts(t, 128)
                    for e in range(2):
                        esl = slice(64 * e, 64 * e + 64)
                        nc.tensor.matmul(
                            sc[:, bass.ts(e, SEQT)],
                            lhsT=kt_sb[esl, hp, ksl],
                            rhs=qt_sb[esl, hp, qsl],
                            start=True,
                            stop=True,
                        )
                    return sc

                ctx = [
                    cx_ps.tile([D + 1, SEQT], f32, tag="cx", name=f"ctx{e}")
                    for e in range(2)
                ]

                sc_next = sc_mms(0)
                for t in range(KT):
                    if t in fillers:
                        fillers[t]()
                    sc_cur = sc_next
                    if t + 1 < KT:
                        sc_next = sc_mms(t + 1)
                    pt = ptp.tile([128, 2 * SEQT], bf16, tag="pt", name="pt")
                    nc.scalar.activation(pt[:], sc_cur[:], AF.Exp, scale=ISD)
                    if t == 0 and prev is not None:
                        emit_norm(*prev)
                    for e in range(2):
                        nc.tensor.matmul(
                            ctx[e][:],
                            lhsT=v_sb[:, t, 2 * hp + e, :],
                            rhs=pt[:, bass.ts(e, SEQT)],
                            start=(t == 0),
                            stop=(t == KT - 1),
                        )
                return (hp, qb, ctx)

            def proj_kv(st):
                def cb():
                    proj_mms("k", st)
                    proj_mms("v", st)
                    v_transposes(st)
                return cb

            def proj_q(st):
                return lambda: proj_mms("q", st)

            def piece(m):
                return lambda: outproj_piece(m)

            # ---- emission ----
            # Head: K/V/Q projections for seq-tile 0 right behind the input
            # stream; later seq-tiles ladder INSIDE attention unit (0,0) as
            # fillers timed to the stream (sc_t for t>=4s needs kv st s).
            proj_mms("k", 0)
            proj_mms("v", 0)
            v_transposes(0)
            proj_mms("q", 0)

            u = attn_unit(0, 0, fillers={3: proj_kv(1), 7: proj_kv(2),
                                         11: proj_kv(3)})
            u = attn_unit(1, 0, prev=u, fillers={3: proj_q(1)})
            u = attn_unit(0, 1, prev=u, fillers={3: proj_q(2)})
            u = attn_unit(1, 1, prev=u, fillers={2: piece(0), 6: piece(1),
                                                 10: piece(2)})
            u = attn_unit(0, 2, prev=u, fillers={2: piece(3), 6: proj_q(3)})
            u = attn_unit(1, 2, prev=u, fillers={2: piece(4), 6: piece(5),
                                                 10: piece(6), 13: piece(7)})
            u = attn_unit(0, 3, prev=u, fillers={2: piece(8), 6: piece(9)})
            u = attn_unit(1, 3, prev=u, fillers={2: piece(10), 6: piece(11)})
            emit_norm(*u)
            for m in range(12, 16):
                outproj_piece(m)

    return nc


def _get_program():
    global _PROGRAM
    if _PROGRAM is None:
        _PROGRAM = _build_program()
        if not getattr(_PROGRAM, "_waits_split", False):
            _split_excess_waits(_PROGRAM)
    return _PROGRAM


def kernel(query, key, value, Wq, bq, Wk, bk, Wv, bv, Wo, bo):
    from concourse.bass_utils import run_bass_kernel_spmd

    nc = _get_program()

    bf = ml_dtypes.bfloat16
    q3 = np.asarray(query, np.float32)
    k3 = np.asarray(key, np.float32)
    v3 = np.asarray(value, np.float32)
    # per-batch x^T [E, S] -> [KE, 128, S], rounded to bf16 on host (the
    # bf16 matmul rounds its inputs anyway)
    xs = {}
    for b in range(B):
        xs[b] = {
            "xq": np.ascontiguousarray(q3[b].T).astype(bf).reshape(KE, 128, S),
            "xk": np.ascontiguousarray(k3[b].T).astype(bf).reshape(KE, 128, S),
            "xv": np.ascontiguousarray(v3[b].T).astype(bf).reshape(KE, 128, S),
        }

    Wq = np.asarray(Wq, np.float32)
    Wk = np.asarray(Wk, np.float32)
    Wv = np.asarray(Wv, np.float32)
    Wo = np.asarray(Wo, np.float32)
    bqf = np.asarray(bq, np.float32)
    bkf = np.asarray(bk, np.float32)
    bvf = np.asarray(bv, np.float32)

    wmaps = []
    for g in range(GPB):
        rsl = slice(DOUT * g, DOUT * (g + 1))
        wmaps.append(
            {
                # lhsT for the projections: (W_g)^T [E, DOUT] -> [KE,128,DOUT]
                "wq": np.ascontiguousarray(Wq[rsl, :].T).astype(bf).reshape(KE, 128, DOUT),
                "wk": np.ascontiguousarray(Wk[rsl, :].T).astype(bf).reshape(KE, 128, DOUT),
                "wv": np.ascontiguousarray(Wv[rsl, :].T).astype(bf).reshape(KE, 128, DOUT),
                # out-proj rhs: Wo^T rows rsl as [128, 2, E], bf16
                "wo": np.ascontiguousarray(
                    Wo[:, rsl].T.reshape(2, 128, E).transpose(1, 0, 2)
                ).astype(bf),
                "bq": np.ascontiguousarray(bqf[rsl].reshape(2, 128).T),
                "bk": np.ascontiguousarray(bkf[rsl].reshape(2, 128).T),
                "bv": np.ascontiguousarray(bvf[rsl].reshape(2, 128).T),
            }
        )

    in_maps = []
    for c in range(NCORES):
        b, g = c // GPB, c % GPB
        m = dict(xs[b])
        m.update(wmaps[g])
        in_maps.append(m)

    res = run_bass_kernel_spmd(nc, in_maps, list(range(NCORES)), trace=False)
    bof = np.asarray(bo, np.float32)
    full = np.empty((B, S, E), np.float32)
    for b in range(B):
        acc = res.results[b * GPB]["out"].astype(np.float32)
        for g in range(1, GPB):
            acc += res.results[b * GPB + g]["out"].astype(np.float32)
        full[b] = acc + bof[None, :]
    return full
